# revision 13
# baseline (speedup 1.0000x reference)
"""Trainium2 Bass kernel for nn_Attention_82051055223090 (v2).

ViT-style multi-head attention with RoPE on non-CLS tokens:
  qkv = x @ w_qkv + b_qkv ; rope(q,k) ; softmax(q k^T / sqrt(D)) v ; proj.

Pure data-parallel over batch (B=32 -> 4 per core x 8 cores), no collectives.
Matmul operands bf16, accumulation fp32 in PSUM, softmax fp32->bf16.

v2 restructure vs v1: the PE stream is explicitly software-pipelined so the
tensor engine never waits on the exp (ACT) chain -- attention j-steps are
interleaved at matmul granularity with "filler" matmuls (QKV of later pairs,
V/proj of neighbor batches) pulled from a FIFO.  Engine rebalance: exp is
alone on ACT; bias/copy/reciprocal/normalize on DVE; rope multiplies on
GpSimd(Pool); rope partition-swap as 2 strided DMAs (was 4); reciprocal
row-move DMA via gpsimd SWDGE so it never queues behind bulk SP DMAs.
Proj emits out^T ([C, N], bias per-partition) -- host transposes back.

Per-core dataflow per batch:
  xT[c,t]    <- contiguous bf16 DMA (pre-transposed on host)     [768, 577]
  qkT        = w_qkv[:, :1536]^T chunks @ x^T (+b)               [1536, 577]
  rope       in [d, t] layout; 32-row swap via 2 strided SBUF DMAs
  v          = x @ w_qkv[:, 1536:] (+b), packed per head pair as
               [v_even | ones | v_odd]; ones rows make the AV matmul also
               emit the softmax denominator
  scoresT    = kT(stationary) @ qT(moving), per 128-token j-tile [577, 577]
  expT       = exp(0.125 * scoresT)        (ACT, PSUM -> SBUF bf16)
  outT|den   = [v|ones] @ expT             (PSUM fp32, accumulated over j)
  normOutT   = outT * reciprocal(den)      (DVE + swdge row-move)
  outT'      = w_proj^T chunks @ normOutT (+b) -> DMA out^T (fp32 [768,577])
"""

from collections import deque

import numpy as np

B, N, C, H, D = 32, 577, 768, 12, 64
NCORES = 8
NB = B // NCORES          # batches per core
P = 128
KT = C // P               # 6 contraction chunks of 128
NPAIR = H // 2            # 6 head pairs
TOK = [(i * P, min(P, N - i * P)) for i in range((N + P - 1) // P)]  # j tiles
NA = 289                  # q-chunk A = [0:289]
NBW = 288                 # q-chunk B = [289:577]

_cache = {}


def _build():
    from contextlib import ExitStack

    import concourse.tile as tile
    from concourse import bacc, mybir
    from concourse.ap import AP

    f32 = mybir.dt.float32
    bf16 = mybir.dt.bfloat16
    AF = mybir.ActivationFunctionType
    OP = mybir.AluOpType

    nc = bacc.Bacc("TRN2", debug=False, enable_partition_id=False)

    xt_d = nc.dram_tensor("xt", [NB, C, N], bf16, kind="ExternalInput").ap()
    wqkv_d = nc.dram_tensor("w_qkv", [C, 3 * C], bf16, kind="ExternalInput").ap()
    wproj_d = nc.dram_tensor("w_proj", [C, C], bf16, kind="ExternalInput").ap()
    bqk_d = nc.dram_tensor("bqk2", [P, 18], f32, kind="ExternalInput").ap()
    bvb_d = nc.dram_tensor("bvb", [P, C], f32, kind="ExternalInput").ap()
    bpb_d = nc.dram_tensor("bpb", [P, C], f32, kind="ExternalInput").ap()
    cost_d = nc.dram_tensor("cost", [64, N], bf16, kind="ExternalInput").ap()
    sins_d = nc.dram_tensor("sins", [64, N], bf16, kind="ExternalInput").ap()
    out_d = nc.dram_tensor("out", [NB, N, C], f32, kind="ExternalOutput").ap()

    def ap3(base_ap, part_off, elem_off, dims):
        """Raw AP on the same tensor: partition slice + multi-dim free dims."""
        rowstr = base_ap.ap[0][0]
        return AP(
            base_ap.tensor,
            base_ap.offset + part_off * rowstr + elem_off,
            [[rowstr, dims[0]]] + [list(d) for d in dims[1:]],
        )

    def ap_swap(base_ap, part_off, ncols):
        """[[64p, 2], [p, 32]] partition pattern + [1, ncols] free dims:
        two 32-row blocks 64 apart, starting at part_off."""
        rowstr = base_ap.ap[0][0]
        return AP(
            base_ap.tensor,
            base_ap.offset + part_off * rowstr,
            [[64 * rowstr, 2], [rowstr, 32], [1, ncols]],
        )

    with tile.TileContext(nc) as tc, ExitStack() as ctx:
        const = ctx.enter_context(tc.tile_pool(name="const", bufs=1))
        sctp = ctx.enter_context(tc.tile_pool(name="sctp", bufs=2, space="PSUM"))
        avp = ctx.enter_context(tc.tile_pool(name="avp", bufs=2, space="PSUM"))
        ps = ctx.enter_context(tc.tile_pool(name="ps", bufs=2, space="PSUM"))
        sb = ctx.enter_context(tc.tile_pool(name="sb", bufs=1))

        # ---- constants (pre-formatted on host, contiguous DMAs) ----
        # batch-0 x chunks ride between the w chunks so the first V matmuls
        # start ~2us in instead of after the whole const stream; wp/bpB
        # (first needed by proj at the end of phase 0) load last.
        # front loads split across the two HWDGE queues (SP + ACT, which is
        # idle until the first exp): w chunks alternate queues, xt0 on SP,
        # rope tables/biases on ACT, wp/bpB (needed only by proj) last.
        xt0 = []
        w_sb = []
        for k in range(KT):
            w = const.tile([P, 3 * C], bf16, tag=f"w{k}", name=f"w{k}")
            eng = nc.sync if k % 2 == 0 else nc.scalar
            eng.dma_start(w, wqkv_d[k * P:(k + 1) * P, :])
            w_sb.append(w)
            xt = const.tile([P, N], bf16, tag=f"xt0_{k}", name=f"xt0_{k}")
            nc.sync.dma_start(xt, xt_d[0, k * P:(k + 1) * P, :])
            xt0.append(xt)

        cosT = const.tile([P, N], bf16, tag="cosT", name="cosT")
        sinS = const.tile([P, N], bf16, tag="sinS", name="sinS")
        for g in range(2):  # duplicate across the two 64-partition groups
            nc.scalar.dma_start(cosT[g * 64:g * 64 + 64, :], cost_d)
            nc.scalar.dma_start(sinS[g * 64:g * 64 + 64, :], sins_d)

        bqk = const.tile([P, 18], f32, tag="bqk", name="bqk")
        nc.scalar.dma_start(bqk, bqk_d)
        bvB = const.tile([P, C], f32, tag="bvB", name="bvB")
        nc.scalar.dma_start(bvB, bvb_d)

        wp_sb = []
        for k in range(KT):
            wp = const.tile([P, C], bf16, tag=f"wp{k}", name=f"wp{k}")
            nc.scalar.dma_start(wp, wproj_d[k * P:(k + 1) * P, :])
            wp_sb.append(wp)
        bpB = const.tile([P, C], f32, tag="bpB", name="bpB")
        nc.scalar.dma_start(bpB, bpb_d)

        ST = {"xts": {}, "vts": {}, "qf": {}, "no": {}}
        items = deque()

        def pull(n):
            while items and n > 0:
                kind, fn = items.popleft()
                fn()
                if kind == "mm":
                    n -= 1

        def emit_all(unit):
            for _, fn in unit:
                fn()

        # ---------- unit builders (lists of ("mm"|"aux", closure)) ----------
        def u_xt(b):
            out = []
            ST["xts"][b] = [None] * KT
            if b == 0:
                ST["xts"][0] = list(xt0)
                return out

            def mk(k):
                def f():
                    xt = sb.tile([P, N], bf16, tag="xt", bufs=12,
                                 name=f"xt{b}_{k}")
                    nc.sync.dma_start(xt, xt_d[b, k * P:(k + 1) * P, :])
                    ST["xts"][b][k] = xt
                return f
            for k in range(KT):
                out.append(("aux", mk(k)))
            return out

        def u_v(b, it, half):
            out = []
            ts, tsz = TOK[it]
            box = {}

            def mk_mm(k):
                def f():
                    if k == 0:
                        box["pv"] = ps.tile([P, 512], f32, tag="ps",
                                            name=f"pv{b}_{it}_{half}")
                        if half == 0:
                            vt = sb.tile([P, NPAIR * 192], bf16, tag="v",
                                         bufs=11, name=f"v{b}_{it}")
                            ST["vts"].setdefault(b, [None] * 5)
                            ST["vts"][b][it] = vt
                    c0 = 2 * C + half * 384
                    nc.tensor.matmul(
                        box["pv"][0:tsz, 0:384],
                        ST["xts"][b][k][:, ts:ts + tsz],
                        w_sb[k][:, c0:c0 + 384],
                        start=(k == 0), stop=(k == KT - 1))
                return f

            def bias():
                vt = ST["vts"][b][it]
                po = 0 if half == 0 else 576
                dst = ap3(vt[:], 0, po, [tsz, (192, 3), (128, 2), (1, 64)])
                src_ = box["pv"][0:tsz, 0:384].rearrange(
                    "p (a c d) -> p a c d", a=3, c=2)
                bsrc = bvB[0:tsz, half * 384:(half + 1) * 384].rearrange(
                    "p (a c d) -> p a c d", a=3, c=2)
                nc.vector.tensor_tensor(dst, src_, bsrc, OP.add)
                if half == 1:
                    ones = ap3(vt[:], 0, 64, [tsz, (192, NPAIR), (1, 64)])
                    nc.vector.memset(ones, 1.0)

            for k in range(KT):
                out.append(("mm", mk_mm(k)))
            out.append(("aux", bias))
            return out

        def u_qk(b, m):
            """One q or k tile: 12 matmuls + bias + rope chain."""
            out = []
            box = {}

            def mk_mm(k, second):
                def f():
                    if k == 0 and not second:
                        box["pA"] = ps.tile([P, 512], f32, tag="ps",
                                            name=f"pqa{b}_{m}")
                        box["pB"] = ps.tile([P, 512], f32, tag="ps",
                                            name=f"pqb{b}_{m}")
                    if not second:
                        nc.tensor.matmul(
                            box["pA"][:, 0:NA], w_sb[k][:, m * P:(m + 1) * P],
                            ST["xts"][b][k][:, 0:NA],
                            start=(k == 0), stop=(k == KT - 1))
                    else:
                        nc.tensor.matmul(
                            box["pB"][:, 0:NBW], w_sb[k][:, m * P:(m + 1) * P],
                            ST["xts"][b][k][:, NA:N],
                            start=(k == 0), stop=(k == KT - 1))
                return f

            def bias_a():
                box["qb"] = sb.tile([P, N], bf16, tag="qb", bufs=4,
                                    name=f"qb{b}_{m}")
                nc.vector.tensor_scalar(
                    box["qb"][:, 0:NA], box["pA"][:, 0:NA],
                    bqk[:, m:m + 1], None, OP.add)

            def bias_b():
                nc.vector.tensor_scalar(
                    box["qb"][:, NA:N], box["pB"][:, 0:NBW],
                    bqk[:, m:m + 1], None, OP.add)

            def rope_ut():
                box["ut"] = sb.tile([P, N], bf16, tag="ut", bufs=4,
                                    name=f"ut{b}_{m}")
                nc.vector.tensor_tensor(box["ut"][:], box["qb"][:], sinS[:],
                                        OP.mult)

            def rope_swap():
                # us[p] = ut[p ^ 32]: four 32-row block moves
                box["us"] = sb.tile([P, N], bf16, tag="us", bufs=4,
                                    name=f"us{b}_{m}")
                for blk in range(4):
                    o0, i0 = blk * 32, (blk ^ 1) * 32
                    nc.sync.dma_start(box["us"][o0:o0 + 32, :],
                                      box["ut"][i0:i0 + 32, :])

            def rope_cos():
                qf = sb.tile([P, N], bf16, tag="qf", bufs=10,
                             name=f"qf{b}_{m}")
                box["qf"] = qf
                ST["qf"][(b, m)] = qf
                nc.vector.tensor_tensor(qf[:], box["qb"][:], cosT[:], OP.mult)

            def rope_add():
                nc.vector.tensor_tensor(box["qf"][:], box["qf"][:],
                                        box["us"][:], OP.add)

            for k in range(KT):
                out.append(("mm", mk_mm(k, False)))
            out.append(("aux", bias_a))
            for k in range(KT):
                out.append(("mm", mk_mm(k, True)))
            out.append(("aux", bias_b))
            out.append(("aux", rope_cos))
            out.append(("aux", rope_ut))
            out.append(("aux", rope_swap))
            out.append(("aux", rope_add))
            return out

        def u_proj(b, it):
            """Token-major proj tile it (v1 style): out[ts:ts+tsz, :]."""
            out = []
            ts, tsz = TOK[it]
            box = {}

            def mk_mm(kk, half):
                def f():
                    if kk == 0:
                        box[half] = ps.tile([P, 512], f32, tag="ps",
                                            name=f"pp{b}_{it}_{half}")
                    c0 = half * 384
                    nc.tensor.matmul(
                        box[half][0:tsz, 0:384],
                        ST["no"][b][kk][:, ts:ts + tsz],
                        wp_sb[kk][:, c0:c0 + 384],
                        start=(kk == 0), stop=(kk == KT - 1))
                return f

            def bias(half):
                def f():
                    if half == 0:
                        box["ot"] = sb.tile([P, C], f32, tag="ot", bufs=3,
                                            name=f"ot{b}_{it}")
                    c0 = half * 384
                    nc.vector.tensor_tensor(
                        box["ot"][0:tsz, c0:c0 + 384], box[half][0:tsz, 0:384],
                        bpB[0:tsz, c0:c0 + 384], OP.add)
                return f

            def dma_out():
                # final batch: alternate queues so the tail out-DMA drain
                # (strictly after the last PE work) runs on both HWDGE
                # queues in parallel
                eng = nc.scalar if (b == NB - 1 and it % 2 == 1) else nc.sync
                eng.dma_start(out_d[b, ts:ts + tsz, :],
                              box["ot"][0:tsz, :])

            for half in range(2):
                for kk in range(KT):
                    out.append(("mm", mk_mm(kk, half)))
                out.append(("aux", bias(half)))
            out.append(("aux", dma_out))
            return out

        # ---------- attention (explicitly pipelined with pulls) ----------
        def emit_att(b, p):
            qft = ST["qf"][(b, p)]
            kft = ST["qf"][(b, 6 + p)]
            vts = ST["vts"][b]
            no_t = sb.tile([P, N], bf16, tag="no", bufs=12, name=f"no{b}_{p}")

            for half in range(2):
                h0 = half * 64
                avA = avp.tile([P, 512], f32, tag="av", name=f"avA{b}_{p}_{half}")
                avB = avp.tile([P, 512], f32, tag="av", name=f"avB{b}_{p}_{half}")
                scts = [None] * 5
                ets = [None] * 5

                def s_step(j):
                    js, jsz = TOK[j]
                    sct = sctp.tile([P, 1024], f32, tag="sc",
                                    name=f"sc{b}_{p}_{half}_{j}")
                    scts[j] = sct
                    nc.tensor.matmul(
                        sct[0:jsz, 0:NA], kft[h0:h0 + 64, js:js + jsz],
                        qft[h0:h0 + 64, 0:NA], skip_group_check=True)
                    nc.tensor.matmul(
                        sct[0:jsz, 512:512 + NA], kft[h0:h0 + 64, js:js + jsz],
                        qft[h0:h0 + 64, NA - 1:N], skip_group_check=True)

                def e_step(j):
                    js, jsz = TOK[j]
                    et = sb.tile([P, 2 * NA], bf16, tag="e", bufs=8,
                                 name=f"e{b}_{p}_{half}_{j}")
                    ets[j] = et
                    nc.scalar.activation(
                        et[0:jsz].rearrange("p (a q) -> p a q", a=2),
                        scts[j][0:jsz].rearrange("p (a q) -> p a q", a=2)[:, :, 0:NA],
                        AF.Exp, scale=0.125)

                def a_step(j):
                    js, jsz = TOK[j]
                    vslice = vts[j][0:jsz, p * 192 + h0:p * 192 + h0 + 128]
                    nc.tensor.matmul(
                        avA[:, 0:NA], vslice, ets[j][0:jsz, 0:NA],
                        start=(j == 0), stop=(j == 4), skip_group_check=True)
                    nc.tensor.matmul(
                        avB[:, 0:NBW], vslice, ets[j][0:jsz, NA + 1:2 * NA],
                        start=(j == 0), stop=(j == 4), skip_group_check=True)

                s_step(0); e_step(0)
                s_step(1); e_step(1)
                pull(6); pull(6)
                a_step(0)
                for j in (2, 3, 4):
                    s_step(j); e_step(j)
                    pull(2)
                    a_step(j - 2)
                pull(1)
                a_step(3)
                pull(1)
                a_step(4)

                # normalize: reciprocal of denom rows, row-move via swdge,
                # multiply into no_t
                drow = 64 - h0
                rec = sb.tile([P, N], f32, tag="rec", bufs=4,
                              name=f"rec{b}_{p}_{half}")
                # full-128-partition recip: reciprocal_approx_fast
                # mis-executes on a base-partition-64 slice, so compute all
                # rows from offset 0 and use only the denominator rows (the
                # data-row outputs are discarded; the row-move DMA + mults
                # below only touch the den-derived rows)
                nc.vector.reciprocal_approx_fast(
                    out=rec[0:P, 0:NA], in_=avA[0:P, 0:NA])
                nc.vector.reciprocal_approx_fast(
                    out=rec[0:P, NA:N], in_=avB[0:P, 0:NBW])
                nc.sync.dma_start(rec[h0:h0 + 64, 0:NA],
                                  rec[drow:drow + 64, 0:NA])
                nc.sync.dma_start(rec[h0:h0 + 64, NA:N],
                                  rec[drow:drow + 64, NA:N])
                nc.vector.tensor_tensor(
                    no_t[h0:h0 + 64, 0:NA], avA[h0:h0 + 64, 0:NA],
                    rec[h0:h0 + 64, 0:NA], OP.mult)
                nc.vector.tensor_tensor(
                    no_t[h0:h0 + 64, NA:N], avB[h0:h0 + 64, 0:NBW],
                    rec[h0:h0 + 64, NA:N], OP.mult)

            ST["no"].setdefault(b, []).append(no_t)

        # ---------- the schedule ----------
        for b in range(NB):
            if b == 0:
                emit_all(u_xt(0))
                for it in range(5):
                    for hf in range(2):
                        emit_all(u_v(0, it, hf))
                for m in (0, 6, 1, 7):
                    emit_all(u_qk(0, m))

            # fillers consumed by this batch's attention phase (ordered so
            # each QK(b, p) completes >= ~2 pair-periods before ATT(b, p),
            # and the next batch's front work rides the phase tail)
            for m in (2, 8, 3, 9):
                items.extend(u_qk(b, m))
            if b > 0:
                items.extend(u_proj(b - 1, 0))
            for m in (4, 10):
                items.extend(u_qk(b, m))
            if b > 0:
                items.extend(u_proj(b - 1, 1))
            for m in (5, 11):
                items.extend(u_qk(b, m))
            if b > 0:
                for it in range(2, 5):
                    items.extend(u_proj(b - 1, it))
            if b + 1 < NB:
                items.extend(u_xt(b + 1))
                for it in range(5):
                    for hf in range(2):
                        items.extend(u_v(b + 1, it, hf))
                for m in (0, 6, 1, 7):
                    items.extend(u_qk(b + 1, m))

            for p in range(NPAIR):
                emit_att(b, p)

            # solid block between phases: drain leftovers (keeps PE fed
            # while ACT catches up; completes next batch's front)
            while items:
                items.popleft()[1]()

        for it in range(5):
            emit_all(u_proj(NB - 1, it))

    nc.compile()
    return nc


def _get_nc():
    if "nc" not in _cache:
        _cache["nc"] = _build()
    return _cache["nc"]


def _prep_shared(inputs):
    """Host-side layout prep shared across cores (numpy only)."""
    import ml_dtypes

    bf = ml_dtypes.bfloat16
    w_qkv = np.ascontiguousarray(np.asarray(inputs["w_qkv"], np.float32)).astype(bf)
    w_proj = np.ascontiguousarray(np.asarray(inputs["w_proj"], np.float32)).astype(bf)
    b_qkv = np.asarray(inputs["b_qkv"], np.float32)
    b_proj = np.asarray(inputs["b_proj"], np.float32)
    sin = np.asarray(inputs["rope_sin"], np.float32)  # [576, 64]
    cos = np.asarray(inputs["rope_cos"], np.float32)

    bqk2 = np.ascontiguousarray(b_qkv.reshape(18, P).T)          # [128, 18]
    bvb = np.ascontiguousarray(np.broadcast_to(b_qkv[2 * C:], (P, C)))
    bpb = np.ascontiguousarray(np.broadcast_to(b_proj, (P, C)))

    cost = np.ones((64, N), np.float32)
    cost[:, 1:] = cos.T
    # sinS holds sin at the swapped index with the rotate-half sign pattern:
    # rows 0:32 <- +sin cols 32:64 ; rows 32:64 <- -sin cols 0:32
    sins = np.zeros((64, N), np.float32)
    sins[0:32, 1:] = sin.T[32:64]
    sins[32:64, 1:] = -sin.T[0:32]

    return {
        "w_qkv": w_qkv,
        "w_proj": w_proj,
        "bqk2": bqk2.astype(np.float32),
        "bvb": bvb.astype(np.float32),
        "bpb": bpb.astype(np.float32),
        "cost": cost.astype(bf),
        "sins": sins.astype(bf),
    }


last_results = None


def kernel(**inputs):
    global last_results
    import ml_dtypes

    from concourse.bass_utils import run_bass_kernel_spmd

    nc = _get_nc()
    bf = ml_dtypes.bfloat16
    x = np.asarray(inputs["x"], np.float32)
    # host-side transpose + bf16 cast: [B, N, C] -> [B, C, N]
    xt_all = np.ascontiguousarray(x.transpose(0, 2, 1)).astype(bf)
    shared = _prep_shared(inputs)

    in_maps = []
    for c in range(NCORES):
        m = dict(shared)
        m["xt"] = np.ascontiguousarray(xt_all[c * NB:(c + 1) * NB])
        in_maps.append(m)

    res = run_bass_kernel_spmd(nc, in_maps, core_ids=list(range(NCORES)))
    last_results = res
    return np.concatenate([res.results[c]["out"] for c in range(NCORES)], axis=0)


# revision 14
# speedup vs baseline: 1.1714x; 1.1714x over previous
"""Trainium2 Bass kernel for nn_Attention_82051055223090 (v2).

ViT-style multi-head attention with RoPE on non-CLS tokens:
  qkv = x @ w_qkv + b_qkv ; rope(q,k) ; softmax(q k^T / sqrt(D)) v ; proj.

Pure data-parallel over batch (B=32 -> 4 per core x 8 cores), no collectives.
Matmul operands bf16, accumulation fp32 in PSUM, softmax fp32->bf16.

v2 restructure vs v1: the PE stream is explicitly software-pipelined so the
tensor engine never waits on the exp (ACT) chain -- attention j-steps are
interleaved at matmul granularity with "filler" matmuls (QKV of later pairs,
V/proj of neighbor batches) pulled from a FIFO.  Engine rebalance: exp is
alone on ACT; bias/copy/reciprocal/normalize on DVE; rope multiplies on
GpSimd(Pool); rope partition-swap as 2 strided DMAs (was 4); reciprocal
row-move DMA via gpsimd SWDGE so it never queues behind bulk SP DMAs.
Proj emits out^T ([C, N], bias per-partition) -- host transposes back.

Per-core dataflow per batch:
  xT[c,t]    <- contiguous bf16 DMA (pre-transposed on host)     [768, 577]
  qkT        = w_qkv[:, :1536]^T chunks @ x^T (+b)               [1536, 577]
  rope       in [d, t] layout; 32-row swap via 2 strided SBUF DMAs
  v          = x @ w_qkv[:, 1536:] (+b), packed per head pair as
               [v_even | ones | v_odd]; ones rows make the AV matmul also
               emit the softmax denominator
  scoresT    = kT(stationary) @ qT(moving), per 128-token j-tile [577, 577]
  expT       = exp(0.125 * scoresT)        (ACT, PSUM -> SBUF bf16)
  outT|den   = [v|ones] @ expT             (PSUM fp32, accumulated over j)
  normOutT   = outT * reciprocal(den)      (DVE + swdge row-move)
  outT'      = w_proj^T chunks @ normOutT (+b) -> DMA out^T (fp32 [768,577])
"""

from collections import deque

import numpy as np

B, N, C, H, D = 32, 577, 768, 12, 64
NCORES = 8
NB = B // NCORES          # batches per core
P = 128
KT = C // P               # 6 contraction chunks of 128
NPAIR = H // 2            # 6 head pairs
TOK = [(i * P, min(P, N - i * P)) for i in range((N + P - 1) // P)]  # j tiles
NA = 289                  # q-chunk A = [0:289]
NBW = 288                 # q-chunk B = [289:577]

_cache = {}


def _build():
    from contextlib import ExitStack

    import concourse.tile as tile
    from concourse import bacc, mybir
    from concourse.ap import AP

    f32 = mybir.dt.float32
    bf16 = mybir.dt.bfloat16
    AF = mybir.ActivationFunctionType
    OP = mybir.AluOpType

    nc = bacc.Bacc("TRN2", debug=False, enable_partition_id=False)

    xt_d = nc.dram_tensor("xt", [NB, C, N], bf16, kind="ExternalInput").ap()
    wqkv_d = nc.dram_tensor("w_qkv", [C, 3 * C], bf16, kind="ExternalInput").ap()
    wproj_d = nc.dram_tensor("w_proj", [C, C], bf16, kind="ExternalInput").ap()
    bqk_d = nc.dram_tensor("bqk2", [P, 18], f32, kind="ExternalInput").ap()
    bvb_d = nc.dram_tensor("bvb", [P, C], f32, kind="ExternalInput").ap()
    bpb_d = nc.dram_tensor("bpb", [P, C], f32, kind="ExternalInput").ap()
    cost_d = nc.dram_tensor("cost", [64, N], bf16, kind="ExternalInput").ap()
    sins_d = nc.dram_tensor("sins", [64, N], bf16, kind="ExternalInput").ap()
    out_d = nc.dram_tensor("out", [NB, N, C], f32, kind="ExternalOutput").ap()

    def ap3(base_ap, part_off, elem_off, dims):
        """Raw AP on the same tensor: partition slice + multi-dim free dims."""
        rowstr = base_ap.ap[0][0]
        return AP(
            base_ap.tensor,
            base_ap.offset + part_off * rowstr + elem_off,
            [[rowstr, dims[0]]] + [list(d) for d in dims[1:]],
        )

    def ap_swap(base_ap, part_off, ncols):
        """[[64p, 2], [p, 32]] partition pattern + [1, ncols] free dims:
        two 32-row blocks 64 apart, starting at part_off."""
        rowstr = base_ap.ap[0][0]
        return AP(
            base_ap.tensor,
            base_ap.offset + part_off * rowstr,
            [[64 * rowstr, 2], [rowstr, 32], [1, ncols]],
        )

    with tile.TileContext(nc) as tc, ExitStack() as ctx:
        const = ctx.enter_context(tc.tile_pool(name="const", bufs=1))
        sctp = ctx.enter_context(tc.tile_pool(name="sctp", bufs=2, space="PSUM"))
        avp = ctx.enter_context(tc.tile_pool(name="avp", bufs=2, space="PSUM"))
        ps = ctx.enter_context(tc.tile_pool(name="ps", bufs=2, space="PSUM"))
        sb = ctx.enter_context(tc.tile_pool(name="sb", bufs=1))

        # ---- constants (pre-formatted on host, contiguous DMAs) ----
        # batch-0 x chunks ride between the w chunks so the first V matmuls
        # start ~2us in instead of after the whole const stream; wp/bpB
        # (first needed by proj at the end of phase 0) load last.
        # front loads split across the two HWDGE queues (SP + ACT, which is
        # idle until the first exp): w chunks alternate queues, xt0 on SP,
        # rope tables/biases on ACT, wp/bpB (needed only by proj) last.
        xt0 = []
        w_sb = []
        for k in range(KT):
            w = const.tile([P, 3 * C], bf16, tag=f"w{k}", name=f"w{k}")
            eng = nc.sync if k % 2 == 0 else nc.scalar
            eng.dma_start(w, wqkv_d[k * P:(k + 1) * P, :])
            w_sb.append(w)
            xt = const.tile([P, N], bf16, tag=f"xt0_{k}", name=f"xt0_{k}")
            nc.sync.dma_start(xt, xt_d[0, k * P:(k + 1) * P, :])
            xt0.append(xt)

        cosT = const.tile([P, N], bf16, tag="cosT", name="cosT")
        sinS = const.tile([P, N], bf16, tag="sinS", name="sinS")
        for g in range(2):  # duplicate across the two 64-partition groups
            nc.scalar.dma_start(cosT[g * 64:g * 64 + 64, :], cost_d)
            nc.scalar.dma_start(sinS[g * 64:g * 64 + 64, :], sins_d)

        bqk = const.tile([P, 18], f32, tag="bqk", name="bqk")
        nc.scalar.dma_start(bqk, bqk_d)
        bvB = const.tile([P, C], f32, tag="bvB", name="bvB")
        nc.scalar.dma_start(bvB, bvb_d)

        wp_sb = []
        for k in range(KT):
            wp = const.tile([P, C], bf16, tag=f"wp{k}", name=f"wp{k}")
            nc.scalar.dma_start(wp, wproj_d[k * P:(k + 1) * P, :])
            wp_sb.append(wp)
        bpB = const.tile([P, C], f32, tag="bpB", name="bpB")
        nc.scalar.dma_start(bpB, bpb_d)

        ST = {"xts": {}, "vts": {}, "qf": {}, "no": {}}
        items = deque()

        def pull(n):
            while items and n > 0:
                kind, fn = items.popleft()
                fn()
                if kind == "mm":
                    n -= 1

        def emit_all(unit):
            for _, fn in unit:
                fn()

        # ---------- unit builders (lists of ("mm"|"aux", closure)) ----------
        def u_xt(b):
            out = []
            ST["xts"][b] = [None] * KT
            if b == 0:
                ST["xts"][0] = list(xt0)
                return out

            def mk(k):
                def f():
                    xt = sb.tile([P, N], bf16, tag="xt", bufs=12,
                                 name=f"xt{b}_{k}")
                    nc.sync.dma_start(xt, xt_d[b, k * P:(k + 1) * P, :])
                    ST["xts"][b][k] = xt
                return f
            for k in range(KT):
                out.append(("aux", mk(k)))
            return out

        def u_v(b, it, half):
            out = []
            ts, tsz = TOK[it]
            box = {}

            def mk_mm(k):
                def f():
                    if k == 0:
                        box["pv"] = ps.tile([P, 512], f32, tag="ps",
                                            name=f"pv{b}_{it}_{half}")
                        if half == 0:
                            vt = sb.tile([P, NPAIR * 192], bf16, tag="v",
                                         bufs=11, name=f"v{b}_{it}")
                            ST["vts"].setdefault(b, [None] * 5)
                            ST["vts"][b][it] = vt
                    c0 = 2 * C + half * 384
                    nc.tensor.matmul(
                        box["pv"][0:tsz, 0:384],
                        ST["xts"][b][k][:, ts:ts + tsz],
                        w_sb[k][:, c0:c0 + 384],
                        start=(k == 0), stop=(k == KT - 1))
                return f

            def bias():
                vt = ST["vts"][b][it]
                po = 0 if half == 0 else 576
                dst = ap3(vt[:], 0, po, [tsz, (192, 3), (128, 2), (1, 64)])
                src_ = box["pv"][0:tsz, 0:384].rearrange(
                    "p (a c d) -> p a c d", a=3, c=2)
                bsrc = bvB[0:tsz, half * 384:(half + 1) * 384].rearrange(
                    "p (a c d) -> p a c d", a=3, c=2)
                nc.vector.tensor_tensor(dst, src_, bsrc, OP.add)
                if half == 1:
                    ones = ap3(vt[:], 0, 64, [tsz, (192, NPAIR), (1, 64)])
                    nc.vector.memset(ones, 1.0)

            for k in range(KT):
                out.append(("mm", mk_mm(k)))
            out.append(("aux", bias))
            return out

        def u_qk(b, m):
            """One q or k tile: 12 matmuls + bias + rope chain."""
            out = []
            box = {}

            def mk_mm(k, second):
                def f():
                    if k == 0 and not second:
                        box["pA"] = ps.tile([P, 512], f32, tag="ps",
                                            name=f"pqa{b}_{m}")
                        box["pB"] = ps.tile([P, 512], f32, tag="ps",
                                            name=f"pqb{b}_{m}")
                    if not second:
                        nc.tensor.matmul(
                            box["pA"][:, 0:NA], w_sb[k][:, m * P:(m + 1) * P],
                            ST["xts"][b][k][:, 0:NA],
                            start=(k == 0), stop=(k == KT - 1))
                    else:
                        nc.tensor.matmul(
                            box["pB"][:, 0:NBW], w_sb[k][:, m * P:(m + 1) * P],
                            ST["xts"][b][k][:, NA:N],
                            start=(k == 0), stop=(k == KT - 1))
                return f

            def bias_a():
                box["qb"] = sb.tile([P, N], bf16, tag="qb", bufs=4,
                                    name=f"qb{b}_{m}")
                nc.vector.tensor_scalar(
                    box["qb"][:, 0:NA], box["pA"][:, 0:NA],
                    bqk[:, m:m + 1], None, OP.add)

            def bias_b():
                nc.vector.tensor_scalar(
                    box["qb"][:, NA:N], box["pB"][:, 0:NBW],
                    bqk[:, m:m + 1], None, OP.add)

            def rope_ut():
                box["ut"] = sb.tile([P, N], bf16, tag="ut", bufs=4,
                                    name=f"ut{b}_{m}")
                nc.vector.tensor_tensor(box["ut"][:], box["qb"][:], sinS[:],
                                        OP.mult)

            def rope_swap():
                # us[p] = ut[p ^ 32]: four 32-row block moves
                box["us"] = sb.tile([P, N], bf16, tag="us", bufs=4,
                                    name=f"us{b}_{m}")
                for blk in range(4):
                    o0, i0 = blk * 32, (blk ^ 1) * 32
                    nc.sync.dma_start(box["us"][o0:o0 + 32, :],
                                      box["ut"][i0:i0 + 32, :])

            def rope_cos():
                qf = sb.tile([P, N], bf16, tag="qf", bufs=10,
                             name=f"qf{b}_{m}")
                box["qf"] = qf
                ST["qf"][(b, m)] = qf
                nc.vector.tensor_tensor(qf[:], box["qb"][:], cosT[:], OP.mult)

            def rope_add():
                nc.vector.tensor_tensor(box["qf"][:], box["qf"][:],
                                        box["us"][:], OP.add)

            for k in range(KT):
                out.append(("mm", mk_mm(k, False)))
            out.append(("aux", bias_a))
            for k in range(KT):
                out.append(("mm", mk_mm(k, True)))
            out.append(("aux", bias_b))
            out.append(("aux", rope_cos))
            out.append(("aux", rope_ut))
            out.append(("aux", rope_swap))
            out.append(("aux", rope_add))
            return out

        def u_proj(b, it):
            """Token-major proj tile it (v1 style): out[ts:ts+tsz, :]."""
            out = []
            ts, tsz = TOK[it]
            box = {}

            def mk_mm(kk, half):
                def f():
                    if kk == 0:
                        box[half] = ps.tile([P, 512], f32, tag="ps",
                                            name=f"pp{b}_{it}_{half}")
                    c0 = half * 384
                    nc.tensor.matmul(
                        box[half][0:tsz, 0:384],
                        ST["no"][b][kk][:, ts:ts + tsz],
                        wp_sb[kk][:, c0:c0 + 384],
                        start=(kk == 0), stop=(kk == KT - 1))
                return f

            def bias(half):
                def f():
                    if half == 0:
                        box["ot"] = sb.tile([P, C], f32, tag="ot", bufs=3,
                                            name=f"ot{b}_{it}")
                    c0 = half * 384
                    nc.vector.tensor_tensor(
                        box["ot"][0:tsz, c0:c0 + 384], box[half][0:tsz, 0:384],
                        bpB[0:tsz, c0:c0 + 384], OP.add)
                return f

            def dma_out():
                nc.sync.dma_start(out_d[b, ts:ts + tsz, :],
                                  box["ot"][0:tsz, :])

            for half in range(2):
                for kk in range(KT):
                    out.append(("mm", mk_mm(kk, half)))
                out.append(("aux", bias(half)))
            out.append(("aux", dma_out))
            return out

        # ---------- attention (explicitly pipelined with pulls) ----------
        def emit_att(b, p):
            qft = ST["qf"][(b, p)]
            kft = ST["qf"][(b, 6 + p)]
            vts = ST["vts"][b]
            no_t = sb.tile([P, N], bf16, tag="no", bufs=12, name=f"no{b}_{p}")

            for half in range(2):
                h0 = half * 64
                avA = avp.tile([P, 512], f32, tag="av", name=f"avA{b}_{p}_{half}")
                avB = avp.tile([P, 512], f32, tag="av", name=f"avB{b}_{p}_{half}")
                scts = [None] * 5
                ets = [None] * 5

                def s_step(j):
                    js, jsz = TOK[j]
                    sct = sctp.tile([P, 1024], f32, tag="sc",
                                    name=f"sc{b}_{p}_{half}_{j}")
                    scts[j] = sct
                    nc.tensor.matmul(
                        sct[0:jsz, 0:NA], kft[h0:h0 + 64, js:js + jsz],
                        qft[h0:h0 + 64, 0:NA], skip_group_check=True)
                    nc.tensor.matmul(
                        sct[0:jsz, 512:512 + NA], kft[h0:h0 + 64, js:js + jsz],
                        qft[h0:h0 + 64, NA - 1:N], skip_group_check=True)

                def e_step(j):
                    js, jsz = TOK[j]
                    et = sb.tile([P, 2 * NA], bf16, tag="e", bufs=8,
                                 name=f"e{b}_{p}_{half}_{j}")
                    ets[j] = et
                    nc.scalar.activation(
                        et[0:jsz].rearrange("p (a q) -> p a q", a=2),
                        scts[j][0:jsz].rearrange("p (a q) -> p a q", a=2)[:, :, 0:NA],
                        AF.Exp, scale=0.125)

                def a_step(j):
                    js, jsz = TOK[j]
                    vslice = vts[j][0:jsz, p * 192 + h0:p * 192 + h0 + 128]
                    nc.tensor.matmul(
                        avA[:, 0:NA], vslice, ets[j][0:jsz, 0:NA],
                        start=(j == 0), stop=(j == 4), skip_group_check=True)
                    nc.tensor.matmul(
                        avB[:, 0:NBW], vslice, ets[j][0:jsz, NA + 1:2 * NA],
                        start=(j == 0), stop=(j == 4), skip_group_check=True)

                s_step(0); e_step(0)
                s_step(1); e_step(1)
                pull(6); pull(6)
                a_step(0)
                for j in (2, 3, 4):
                    s_step(j); e_step(j)
                    pull(2)
                    a_step(j - 2)
                pull(1)
                a_step(3)
                pull(1)
                a_step(4)

                # normalize: reciprocal of denom rows, row-move via swdge,
                # multiply into no_t
                drow = 64 - h0
                rec = sb.tile([P, N], f32, tag="rec", bufs=4,
                              name=f"rec{b}_{p}_{half}")
                # full-128-partition recip: reciprocal_approx_fast
                # mis-executes on a base-partition-64 slice, so compute all
                # rows from offset 0 and use only the denominator rows (the
                # data-row outputs are discarded; the row-move DMA + mults
                # below only touch the den-derived rows)
                nc.vector.reciprocal_approx_fast(
                    out=rec[0:P, 0:NA], in_=avA[0:P, 0:NA])
                nc.vector.reciprocal_approx_fast(
                    out=rec[0:P, NA:N], in_=avB[0:P, 0:NBW])
                nc.sync.dma_start(rec[h0:h0 + 64, 0:NA],
                                  rec[drow:drow + 64, 0:NA])
                nc.sync.dma_start(rec[h0:h0 + 64, NA:N],
                                  rec[drow:drow + 64, NA:N])
                nc.vector.tensor_tensor(
                    no_t[h0:h0 + 64, 0:NA], avA[h0:h0 + 64, 0:NA],
                    rec[h0:h0 + 64, 0:NA], OP.mult)
                nc.vector.tensor_tensor(
                    no_t[h0:h0 + 64, NA:N], avB[h0:h0 + 64, 0:NBW],
                    rec[h0:h0 + 64, NA:N], OP.mult)

            ST["no"].setdefault(b, []).append(no_t)

        # ---------- the schedule ----------
        for b in range(NB):
            if b == 0:
                emit_all(u_xt(0))
                for it in range(5):
                    for hf in range(2):
                        emit_all(u_v(0, it, hf))
                for m in (0, 6, 1, 7):
                    emit_all(u_qk(0, m))

            # fillers consumed by this batch's attention phase (ordered so
            # each QK(b, p) completes >= ~2 pair-periods before ATT(b, p),
            # and the next batch's front work rides the phase tail)
            for m in (2, 8, 3, 9):
                items.extend(u_qk(b, m))
            if b > 0:
                items.extend(u_proj(b - 1, 0))
            for m in (4, 10):
                items.extend(u_qk(b, m))
            if b > 0:
                items.extend(u_proj(b - 1, 1))
            for m in (5, 11):
                items.extend(u_qk(b, m))
            if b > 0:
                for it in range(2, 5):
                    items.extend(u_proj(b - 1, it))
            if b + 1 < NB:
                items.extend(u_xt(b + 1))
                for it in range(5):
                    for hf in range(2):
                        items.extend(u_v(b + 1, it, hf))
                for m in (0, 6, 1, 7):
                    items.extend(u_qk(b + 1, m))

            for p in range(NPAIR):
                emit_att(b, p)

            # solid block between phases: drain leftovers (keeps PE fed
            # while ACT catches up; completes next batch's front)
            while items:
                items.popleft()[1]()

        for it in range(5):
            emit_all(u_proj(NB - 1, it))

    nc.compile()
    return nc


def _get_nc():
    if "nc" not in _cache:
        _cache["nc"] = _build()
    return _cache["nc"]


def _prep_shared(inputs):
    """Host-side layout prep shared across cores (numpy only)."""
    import ml_dtypes

    bf = ml_dtypes.bfloat16
    w_qkv = np.ascontiguousarray(np.asarray(inputs["w_qkv"], np.float32)).astype(bf)
    w_proj = np.ascontiguousarray(np.asarray(inputs["w_proj"], np.float32)).astype(bf)
    b_qkv = np.asarray(inputs["b_qkv"], np.float32)
    b_proj = np.asarray(inputs["b_proj"], np.float32)
    sin = np.asarray(inputs["rope_sin"], np.float32)  # [576, 64]
    cos = np.asarray(inputs["rope_cos"], np.float32)

    bqk2 = np.ascontiguousarray(b_qkv.reshape(18, P).T)          # [128, 18]
    bvb = np.ascontiguousarray(np.broadcast_to(b_qkv[2 * C:], (P, C)))
    bpb = np.ascontiguousarray(np.broadcast_to(b_proj, (P, C)))

    cost = np.ones((64, N), np.float32)
    cost[:, 1:] = cos.T
    # sinS holds sin at the swapped index with the rotate-half sign pattern:
    # rows 0:32 <- +sin cols 32:64 ; rows 32:64 <- -sin cols 0:32
    sins = np.zeros((64, N), np.float32)
    sins[0:32, 1:] = sin.T[32:64]
    sins[32:64, 1:] = -sin.T[0:32]

    return {
        "w_qkv": w_qkv,
        "w_proj": w_proj,
        "bqk2": bqk2.astype(np.float32),
        "bvb": bvb.astype(np.float32),
        "bpb": bpb.astype(np.float32),
        "cost": cost.astype(bf),
        "sins": sins.astype(bf),
    }


last_results = None


def kernel(**inputs):
    global last_results
    import ml_dtypes

    from concourse.bass_utils import run_bass_kernel_spmd

    nc = _get_nc()
    bf = ml_dtypes.bfloat16
    x = np.asarray(inputs["x"], np.float32)
    # host-side transpose + bf16 cast: [B, N, C] -> [B, C, N]
    xt_all = np.ascontiguousarray(x.transpose(0, 2, 1)).astype(bf)
    shared = _prep_shared(inputs)

    in_maps = []
    for c in range(NCORES):
        m = dict(shared)
        m["xt"] = np.ascontiguousarray(xt_all[c * NB:(c + 1) * NB])
        in_maps.append(m)

    res = run_bass_kernel_spmd(nc, in_maps, core_ids=list(range(NCORES)))
    last_results = res
    return np.concatenate([res.results[c]["out"] for c in range(NCORES)], axis=0)
